# revision 11
# baseline (speedup 1.0000x reference)
"""Trainium2 Bass kernel for a dense transformer encoder layer.

Reference computation (per batch b):
    q = x.reshape(L, H, E)                       # H=16 heads, E=64
    scores = q @ q^T per head, scaled softmax    # A = softmax(s/8)
    new_x  = concat_h(A_h @ q_h)                 # [L, D]
    x1 = LN(x + new_x; g1, be1)
    y  = relu(x1 @ w1^T + b1) @ w2^T + b2
    out = LN(x1 + y; g2, be2)

Sharding: pure data parallel over (batch, seq-half): core c handles
batch c//2, query rows [(c%2)*1024, +1024).  Keys/values span the full
sequence of that batch.  No device collectives.

Per-core design (all matmuls bf16):
  - xT (transposed x, [d, s] layout) is built by the DMA xbar transpose
    engine straight from HBM -- zero PE/DVE cost.
  - scores for head pair (2t, 2t+1) run CONCURRENTLY on the PE via row
    tiling (each head is K=64: rows 0-63 / 64-127 of the array), writing
    one [128, 1024] PSUM tile; a single 1024-wide exp on the scalar
    engine covers both heads.
  - AV uses the ones-column trick: stationary [V_h | 1] ([128, 65]) per
    (u, head); the U^T accumulator rides in PSUM per (head, l-slab 512).
  - per (pair, slab) epilogue: U^T tiles are PE-transposed back,
    divided by the rowsum (reciprocal on [128,1], not [1,512]), residual
    added, and bn_stats for LN1 accumulated incrementally.
  - FFN weights are host-repacked so each DMA instruction moves a whole
    f-tile (w1: 256KB, 2KB/partition lines) or d-tile (w2: 1MB,
    8KB/partition lines); each weight byte is loaded once per s-slab.
  - FFN2 accumulates all 32 f-tiles of one (dt, slab) in a single PSUM
    group -- no intermediate adds.
  - x1T and yT transposes also go through the DMA xbar.
"""

import numpy as np

import concourse.bass as bass
import concourse.tile as tile
from concourse import bacc
from concourse import mybir
from concourse.masks import make_identity

F32 = mybir.dt.float32
BF16 = mybir.dt.bfloat16
EXP = mybir.ActivationFunctionType.Exp
RELU = mybir.ActivationFunctionType.Relu
SQRT = mybir.ActivationFunctionType.Sqrt
ADD = mybir.AluOpType.add
SUB = mybir.AluOpType.subtract
MUL = mybir.AluOpType.mult

LN_EPS = 1e-5
E = 64          # head dim
W = E + 1       # head dim + ones column
P = 128         # partitions


def build_program(S=2048, D=1024, F=4096, n_cores_unused=8):
    """Build the per-core Bass program.  S = full seq len, queries are the
    first Lq = S//2 rows of xb."""
    H = D // E      # 16 heads
    NP = H // 2     # 8 head pairs
    Lq = S // 2     # 1024 queries per core
    ST = S // P     # 16 s-tiles (key tiles)
    LT = Lq // P    # 8 query row tiles
    DT = D // P     # 8 d chunks
    FT = F // P     # 32 f tiles
    SL = 512        # l-slab width
    NSL = Lq // SL  # 2 slabs
    LPS = SL // P   # 4 l-tiles per slab

    nc = bacc.Bacc("TRN2")

    xb = nc.dram_tensor("xb", [S, D], F32, kind="ExternalInput")
    xb16 = nc.dram_tensor("xb16", [S, D], BF16, kind="ExternalInput")
    xbaug = nc.dram_tensor("xbaug", [S, H * W], BF16, kind="ExternalInput")
    w1s = nc.dram_tensor("w1s", [P, FT, DT, P], BF16, kind="ExternalInput")
    w2s = nc.dram_tensor("w2s", [P, DT, FT, P], BF16, kind="ExternalInput")
    b1 = nc.dram_tensor("b1", [F], F32, kind="ExternalInput")
    b2 = nc.dram_tensor("b2", [D], F32, kind="ExternalInput")
    g1 = nc.dram_tensor("g1", [D], F32, kind="ExternalInput")
    be1 = nc.dram_tensor("be1", [D], F32, kind="ExternalInput")
    g2 = nc.dram_tensor("g2", [D], F32, kind="ExternalInput")
    be2 = nc.dram_tensor("be2", [D], F32, kind="ExternalInput")
    out = nc.dram_tensor("out", [Lq, D], F32, kind="ExternalOutput")

    def bcast(dram_vec, n):
        a = dram_vec[:]
        return bass.AP(tensor=a.tensor, offset=a.offset, ap=[[0, P]] + a.ap)

    with tile.TileContext(nc) as tc:
        with (
            tc.tile_pool(name="persist", bufs=1) as persist,
            tc.tile_pool(name="small", bufs=6) as small,
            tc.tile_pool(name="gb", bufs=1) as gbp,
        ):
            ident = persist.tile([P, P], F32)
            make_identity(nc, ident)
            b1s = persist.tile([P, FT], F32)
            nc.scalar.dma_start(out=b1s, in_=b1[:].rearrange("(t p) -> p t", p=P))
            b2s = persist.tile([P, DT], F32)
            nc.scalar.dma_start(out=b2s, in_=b2[:].rearrange("(t p) -> p t", p=P))
            epst = persist.tile([P, 1], F32)
            nc.vector.memset(epst, LN_EPS)
            # new_x accumulates attention output, then becomes x1 (LN1 in
            # place), then res2 base.
            new_x = persist.tile([P, LT, D], F32)
            # incremental bn_stats for LN1: one record per (l-tile, pair)
            st1 = persist.tile([P, LT, NP, 6], F32)

            # ---------------- attention ----------------
            with (
                tc.tile_pool(name="attn_sb", bufs=1) as asb,
                tc.tile_pool(name="etp", bufs=3) as etp,
                tc.tile_pool(name="utsp", bufs=4) as utsp,
                tc.tile_pool(name="scp", bufs=2, space="PSUM") as scp,
                tc.tile_pool(name="utp", bufs=2, space="PSUM") as utp,
                tc.tile_pool(name="tpp", bufs=2, space="PSUM") as tpp,
            ):
                # x^T via the DMA xbar transpose engine: chunk t holds
                # heads 2t (partitions 0-63) and 2t+1 (partitions 64-127).
                # First on the sync queue: the whole kernel starts here.
                xT = asb.tile([P, DT, S], BF16)
                for t in range(DT):
                    nc.sync.dma_start(
                        out=xT[:, t, :],
                        in_=xb16[:, t * P:(t + 1) * P],
                        transpose=True)

                # values interleaved with ones columns, host-prepacked:
                # [P, ST, H, W]; issued on the scalar queue so they don't
                # delay the xT transposes.
                vaug = asb.tile([P, ST, H * W], BF16)
                nc.scalar.dma_start(
                    out=vaug,
                    in_=xbaug[:, :].rearrange("(u p) hw -> p u hw", p=P))
                vaug = vaug.rearrange("p u (h w) -> p u h w", w=W)

                # residual rows (queries only), fp32: one DMA.
                xres = asb.tile([P, LT, D], F32)
                nc.scalar.dma_start(
                    out=xres,
                    in_=xb[0:Lq, :].rearrange("(l p) d -> p l d", p=P))

                def make_epilogue(t, s, utA, utB):
                    """Chunked epilogue for (pair, slab): returns a list of
                    closures, sprinkled across the next slab's u-loop so the
                    DVE-gated PE transposes never bubble the PE."""
                    hA, hB = 2 * t, 2 * t + 1
                    utss = {}

                    def copies():
                        for h, ut in ((hA, utA), (hB, utB)):
                            uts = utsp.tile([W, SL], F32, tag="uts",
                                            name="uts")
                            nc.vector.tensor_copy(out=uts, in_=ut)
                            utss[h] = uts

                    def tp_chunk(h, j):
                        def run():
                            lt = s * LPS + j
                            up = tpp.tile([P, W], F32, tag="tp", name="up")
                            nc.tensor.transpose(
                                up, utss[h][:, j * P:(j + 1) * P],
                                ident[0:W, 0:W])
                            nc.vector.reciprocal(
                                out=up[:, E:W], in_=up[:, E:W])
                            nc.vector.tensor_scalar_mul(
                                out=new_x[:, lt, h * E:(h + 1) * E],
                                in0=up[:, 0:E], scalar1=up[:, E:W])
                        return run

                    def res_chunk(j):
                        def run():
                            lt = s * LPS + j
                            cols = slice(t * P, (t + 1) * P)
                            nc.vector.tensor_add(
                                out=new_x[:, lt, cols],
                                in0=new_x[:, lt, cols],
                                in1=xres[:, lt, cols])
                            nc.vector.bn_stats(
                                out=st1[:, lt, t, :],
                                in_=new_x[:, lt, cols])
                        return run

                    steps = [copies]
                    for j in range(LPS):
                        steps.append(tp_chunk(hA, j))
                        steps.append(tp_chunk(hB, j))
                    for j in range(LPS):
                        steps.append(res_chunk(j))
                    return steps

                pend_epi = []
                for t in range(NP):
                    hA, hB = 2 * t, 2 * t + 1
                    for s in range(NSL):
                        ls = slice(s * SL, (s + 1) * SL)
                        utA = utp.tile([W, SL], F32, tag="ut")
                        utB = utp.tile([W, SL], F32, tag="ut")
                        pend_av = None
                        for u in range(ST):
                            sc = scp.tile([P, 2 * SL], F32)
                            et = etp.tile([P, 2 * SL], BF16)
                            ku = slice(u * P, (u + 1) * P)
                            # paired scores: rows 0-63 (head A) and 64-127
                            # (head B) of the PE run concurrently.
                            nc.tensor.matmul(
                                sc[:, 0:SL],
                                xT[0:E, t, ku], xT[0:E, t, ls],
                                start=True, stop=True)
                            nc.tensor.matmul(
                                sc[:, SL:2 * SL],
                                xT[E:P, t, ku], xT[E:P, t, ls],
                                start=True, stop=True)
                            nc.scalar.activation(
                                out=et, in_=sc, func=EXP, scale=1.0 / 8.0)
                            # drip the previous slab's epilogue into this
                            # u-loop (one step per u, starting at u=1)
                            if u >= 1 and pend_epi:
                                pend_epi.pop(0)()
                            if pend_av is not None:
                                eo, uo = pend_av
                                nc.tensor.matmul(
                                    utA, vaug[:, uo, hA, :], eo[:, 0:SL],
                                    start=(uo == 0), stop=(uo == ST - 1))
                                nc.tensor.matmul(
                                    utB, vaug[:, uo, hB, :],
                                    eo[:, SL:2 * SL],
                                    start=(uo == 0), stop=(uo == ST - 1))
                            pend_av = (et, u)
                        eo, uo = pend_av
                        nc.tensor.matmul(
                            utA, vaug[:, uo, hA, :], eo[:, 0:SL],
                            start=(uo == 0), stop=(uo == ST - 1))
                        nc.tensor.matmul(
                            utB, vaug[:, uo, hB, :], eo[:, SL:2 * SL],
                            start=(uo == 0), stop=(uo == ST - 1))
                        for step in pend_epi:  # leftovers, if any
                            step()
                        pend_epi = make_epilogue(t, s, utA, utB)
                for step in pend_epi:
                    step()

                # residual 1 + LN1 (in place on new_x)
                g1b = gbp.tile([P, D], F32, tag="g")
                nc.gpsimd.dma_start(out=g1b, in_=bcast(g1, D))
                be1b = gbp.tile([P, D], F32, tag="be")
                nc.gpsimd.dma_start(out=be1b, in_=bcast(be1, D))

            # ---------------- FFN ----------------
            with (
                tc.tile_pool(name="ffn_sb", bufs=1) as fsb,
                tc.tile_pool(name="htp", bufs=33) as htp,
                tc.tile_pool(name="w1p", bufs=3) as w1p,
                tc.tile_pool(name="w2p", bufs=2) as w2p,
                tc.tile_pool(name="ytp", bufs=3) as ytp,
                tc.tile_pool(name="ytTp", bufs=2) as ytTp,
                tc.tile_pool(name="outp", bufs=2) as outp,
                tc.tile_pool(name="ydp", bufs=1, space="DRAM") as ydp,
                tc.tile_pool(name="hpp", bufs=3, space="PSUM") as hpp,
                tc.tile_pool(name="ypp", bufs=3, space="PSUM") as ypp,
                tc.tile_pool(name="x1tp", bufs=2, space="PSUM") as x1tp,
            ):
                x1T = fsb.tile([P, DT, Lq], BF16)
                ydram = ydp.tile([D, Lq], BF16)
                st2 = fsb.tile([P, LT, 2, 6], F32)
                rstds = fsb.tile([P, LT], F32)

                def ln1_block(lts):
                    """Normalize (no affine: g1/be1 are folded into w1/b1
                    on the host) + PE-transpose to x1T.  Runs while the PE
                    is otherwise idle, so keep the chain minimal."""
                    for lt in lts:
                        mv = small.tile([P, 2], F32, tag="bnmv", name="mv")
                        nc.vector.bn_aggr(out=mv, in_=st1[:, lt, :, :])
                        nc.scalar.activation(
                            out=rstds[:, lt:lt + 1], in_=mv[:, 1:2],
                            func=SQRT, bias=epst)
                        nc.vector.reciprocal(
                            out=rstds[:, lt:lt + 1], in_=rstds[:, lt:lt + 1])
                        nc.vector.tensor_scalar(
                            out=new_x[:, lt, :], in0=new_x[:, lt, :],
                            scalar1=mv[:, 0:1], scalar2=rstds[:, lt:lt + 1],
                            op0=SUB, op1=MUL)
                        for c in range(DT):
                            tp = x1tp.tile([P, P], F32, name="tp")
                            nc.tensor.transpose(
                                tp, new_x[:, lt, c * P:(c + 1) * P], ident)
                            cp = (nc.scalar.copy if (c % 2) else
                                  nc.vector.tensor_copy)
                            cp(out=x1T[:, c, lt * P:(lt + 1) * P], in_=tp)

                def ln1_affine(lts):
                    """Apply x1 = z*g1 + be1 in place on new_x (needed only
                    for the residual-2 path); runs while the PE crunches
                    the FFN."""
                    for lt in lts:
                        nc.vector.tensor_mul(
                            out=new_x[:, lt, :], in0=new_x[:, lt, :],
                            in1=g1b)
                        nc.vector.tensor_add(
                            out=new_x[:, lt, :], in0=new_x[:, lt, :],
                            in1=be1b)

                def ffn1(s):
                    ls = slice(s * SL, (s + 1) * SL)
                    hts = []
                    for ft in range(FT):
                        w1t = w1p.tile([P, DT, P], BF16, tag="w1",
                                       name="w1t")
                        nc.sync.dma_start(out=w1t, in_=w1s[:, ft, :, :])
                        hp = hpp.tile([P, SL], F32, name="hp")
                        for dc in range(DT):
                            nc.tensor.matmul(
                                hp, w1t[:, dc, :], x1T[:, dc, ls],
                                start=(dc == 0), stop=(dc == DT - 1))
                        ht = htp.tile([P, SL], BF16, tag="ht", name="ht")
                        nc.scalar.activation(
                            out=ht, in_=hp, func=RELU,
                            bias=b1s[:, ft:ft + 1])
                        hts.append(ht)
                    return hts

                def ffn2(s, hts):
                    ls = slice(s * SL, (s + 1) * SL)
                    for dt in range(DT):
                        w2t = w2p.tile([P, FT, P], BF16, tag="w2",
                                       name="w2t")
                        nc.sync.dma_start(out=w2t, in_=w2s[:, dt, :, :])
                        yp = ypp.tile([P, SL], F32, name="yp")
                        for j in range(FT):
                            nc.tensor.matmul(
                                yp, w2t[:, j, :], hts[j],
                                start=(j == 0), stop=(j == FT - 1))
                        yTb = ytp.tile([P, SL], BF16, tag="yT", name="yTb")
                        nc.vector.tensor_scalar_add(
                            out=yTb, in0=yp, scalar1=b2s[:, dt:dt + 1])
                        nc.sync.dma_start(
                            out=ydram[dt * P:(dt + 1) * P, ls], in_=yTb)

                def tail(s):
                    # y^T back to row-major via one big xbar transpose per
                    # l-tile, issued on the scalar queue; then res2 + LN2.
                    for j in range(LPS):
                        lt = s * LPS + j
                        ytT = ytTp.tile([P, D], BF16, tag="ytT", name="ytT")
                        nc.scalar.dma_start(
                            out=ytT, in_=ydram[:, lt * P:(lt + 1) * P],
                            transpose=True)
                        nc.vector.tensor_add(
                            out=new_x[:, lt, :], in0=new_x[:, lt, :],
                            in1=ytT)
                        nc.vector.bn_stats(
                            out=st2[:, lt, 0, :], in_=new_x[:, lt, 0:D // 2])
                        nc.vector.bn_stats(
                            out=st2[:, lt, 1, :], in_=new_x[:, lt, D // 2:D])
                        ot = outp.tile([P, D], F32, name="ot")
                        _ln_apply(nc, small, new_x[:, lt, :],
                                  st2[:, lt, :, :], g2b, be2b, epst,
                                  out_ap=ot)
                        nc.sync.dma_start(
                            out=out[lt * P:(lt + 1) * P, :], in_=ot)

                ln1_block(range(0, LPS))
                hts0 = ffn1(0)
                ln1_block(range(LPS, LT))
                ln1_affine(range(0, LPS))
                g2b = gbp.tile([P, D], F32, tag="g")
                nc.gpsimd.dma_start(out=g2b, in_=bcast(g2, D))
                be2b = gbp.tile([P, D], F32, tag="be")
                nc.gpsimd.dma_start(out=be2b, in_=bcast(be2, D))
                ffn2(0, hts0)
                hts1 = ffn1(1)
                ln1_affine(range(LPS, LT))
                tail(0)
                ffn2(1, hts1)
                tail(1)

    nc.finalize()
    return nc


def _ln_apply(nc, small, x_ap, st_ap, gb, beb, epst, out_ap=None):
    """LayerNorm apply given pre-computed bn_stats records st_ap
    ([P, ngroups, 6]).  In place on x_ap unless out_ap given."""
    if out_ap is None:
        out_ap = x_ap
    mv = small.tile([P, 2], F32, tag="bnmv")
    nc.vector.bn_aggr(out=mv, in_=st_ap)
    rstd = small.tile([P, 1], F32, tag="rstd")
    nc.scalar.activation(out=rstd, in_=mv[:, 1:2], func=SQRT, bias=epst)
    nc.vector.reciprocal(out=rstd, in_=rstd)
    nc.vector.tensor_scalar(
        out=x_ap, in0=x_ap, scalar1=mv[:, 0:1], scalar2=rstd,
        op0=SUB, op1=MUL)
    nc.vector.tensor_mul(out=x_ap, in0=x_ap, in1=gb)
    nc.vector.tensor_add(out=out_ap, in0=x_ap, in1=beb)


# ---------------------------------------------------------------------------
# host side
# ---------------------------------------------------------------------------

_PROG_CACHE = {}


def get_program(S=2048, D=1024, F=4096):
    key = (S, D, F)
    if key not in _PROG_CACHE:
        _PROG_CACHE[key] = build_program(S, D, F)
    return _PROG_CACHE[key]


def make_in_maps(x, w1, b1, w2, b2, g1, be1, g2, be2, n_cores=8):
    B, L, D = x.shape
    F = w1.shape[0]
    H = D // E
    Lq = L // 2
    DT, FT = D // 128, F // 128
    import ml_dtypes
    BF = ml_dtypes.bfloat16
    # Fold LN1's affine into the first FFN layer: the device feeds the
    # *normalized* x into FFN1 and applies g1/be1 only on the residual
    # path.  relu(w1 @ (z*g1 + be1) + b1) == relu((w1*g1) @ z + (b1 + w1@be1))
    b1 = b1 + w1 @ be1
    w1 = w1 * g1[None, :]
    # w1s[p_d, ft, dc, p_f] so one DMA per f-tile has 2KB/partition lines
    w1s = np.ascontiguousarray(
        w1.T.reshape(DT, 128, FT, 128).transpose(1, 2, 0, 3)).astype(BF)
    # w2s[p_f, dt, ft, p_d] so one DMA per d-tile has 8KB/partition lines
    w2s = np.ascontiguousarray(
        w2.T.reshape(FT, 128, DT, 128).transpose(1, 2, 0, 3)).astype(BF)
    common = dict(w1s=w1s, w2s=w2s, b1=b1, b2=b2, g1=g1, be1=be1,
                  g2=g2, be2=be2)
    in_maps = []
    for c in range(n_cores):
        b, half = c // 2, c % 2
        lo = half * Lq
        xq = x[b, lo:lo + Lq]
        xo = x[b, Lq - lo:2 * Lq - lo]
        xbl = np.ascontiguousarray(np.concatenate([xq, xo], axis=0))
        xbl16 = xbl.astype(BF)
        # values + ones column per head, host-baked: [S, H, W]
        aug = np.empty((L, H, E + 1), dtype=BF)
        aug[:, :, 0:E] = xbl16.reshape(L, H, E)
        aug[:, :, E] = BF(1.0)
        in_maps.append(dict(xb=xbl, xb16=xbl16,
                            xbaug=np.ascontiguousarray(aug.reshape(L, -1)),
                            **common))
    return in_maps


def kernel(x, w1, b1, w2, b2, g1, be1, g2, be2):
    from concourse.bass_utils import run_bass_kernel_spmd

    x = np.asarray(x, dtype=np.float32)
    B, L, D = x.shape
    F = w1.shape[0]
    Lq = L // 2
    n_cores = 2 * B
    nc = get_program(L, D, F)
    in_maps = make_in_maps(x, np.asarray(w1, np.float32), np.asarray(b1, np.float32),
                           np.asarray(w2, np.float32), np.asarray(b2, np.float32),
                           np.asarray(g1, np.float32), np.asarray(be1, np.float32),
                           np.asarray(g2, np.float32), np.asarray(be2, np.float32),
                           n_cores)
    res = run_bass_kernel_spmd(nc, in_maps, core_ids=list(range(n_cores)))
    outp = np.empty((B, L, D), dtype=np.float32)
    for c in range(n_cores):
        b, half = c // 2, c % 2
        outp[b, half * Lq:(half + 1) * Lq] = res.results[c]["out"]
    return outp


# revision 15
# speedup vs baseline: 1.0352x; 1.0352x over previous
"""Trainium2 Bass kernel for a dense transformer encoder layer.

Reference computation (per batch b):
    q = x.reshape(L, H, E)                       # H=16 heads, E=64
    scores = q @ q^T per head, scaled softmax    # A = softmax(s/8)
    new_x  = concat_h(A_h @ q_h)                 # [L, D]
    x1 = LN(x + new_x; g1, be1)
    y  = relu(x1 @ w1^T + b1) @ w2^T + b2
    out = LN(x1 + y; g2, be2)

Sharding: pure data parallel over (batch, seq-half): core c handles
batch c//2, query rows [(c%2)*1024, +1024).  Keys/values span the full
sequence of that batch.  No device collectives.

Per-core design (all matmuls bf16):
  - xT (transposed x, [d, s] layout) is built by the DMA xbar transpose
    engine straight from HBM -- zero PE/DVE cost.
  - scores for head pair (2t, 2t+1) run CONCURRENTLY on the PE via row
    tiling (each head is K=64: rows 0-63 / 64-127 of the array), writing
    one [128, 1024] PSUM tile; a single 1024-wide exp on the scalar
    engine covers both heads.
  - AV uses the ones-column trick: stationary [V_h | 1] ([128, 65]) per
    (u, head); the U^T accumulator rides in PSUM per (head, l-slab 512).
  - per (pair, slab) epilogue: U^T tiles are PE-transposed back,
    divided by the rowsum (reciprocal on [128,1], not [1,512]), residual
    added, and bn_stats for LN1 accumulated incrementally.
  - FFN weights are host-repacked so each DMA instruction moves a whole
    f-tile (w1: 256KB, 2KB/partition lines) or d-tile (w2: 1MB,
    8KB/partition lines); each weight byte is loaded once per s-slab.
  - FFN2 accumulates all 32 f-tiles of one (dt, slab) in a single PSUM
    group -- no intermediate adds.
  - x1T and yT transposes also go through the DMA xbar.
"""

import numpy as np

import concourse.bass as bass
import concourse.tile as tile
from concourse import bacc
from concourse import mybir
from concourse.masks import make_identity

F32 = mybir.dt.float32
BF16 = mybir.dt.bfloat16
EXP = mybir.ActivationFunctionType.Exp
RELU = mybir.ActivationFunctionType.Relu
SQRT = mybir.ActivationFunctionType.Sqrt
ADD = mybir.AluOpType.add
SUB = mybir.AluOpType.subtract
MUL = mybir.AluOpType.mult

LN_EPS = 1e-5
E = 64          # head dim
W = E + 1       # head dim + ones column
P = 128         # partitions


def build_program(S=2048, D=1024, F=4096, n_cores_unused=8):
    """Build the per-core Bass program.  S = full seq len, queries are the
    first Lq = S//2 rows of xb."""
    H = D // E      # 16 heads
    NP = H // 2     # 8 head pairs
    Lq = S // 2     # 1024 queries per core
    ST = S // P     # 16 s-tiles (key tiles)
    LT = Lq // P    # 8 query row tiles
    DT = D // P     # 8 d chunks
    FT = F // P     # 32 f tiles
    SL = 512        # l-slab width
    NSL = Lq // SL  # 2 slabs
    LPS = SL // P   # 4 l-tiles per slab

    nc = bacc.Bacc("TRN2")

    xb = nc.dram_tensor("xb", [S, D], F32, kind="ExternalInput")
    xb16 = nc.dram_tensor("xb16", [S, D], BF16, kind="ExternalInput")
    xbaug = nc.dram_tensor("xbaug", [S, H * W], BF16, kind="ExternalInput")
    w1s = nc.dram_tensor("w1s", [P, FT, DT, P], BF16, kind="ExternalInput")
    w2s = nc.dram_tensor("w2s", [P, DT, FT, P], BF16, kind="ExternalInput")
    b1 = nc.dram_tensor("b1", [F], F32, kind="ExternalInput")
    b2 = nc.dram_tensor("b2", [D], F32, kind="ExternalInput")
    g1 = nc.dram_tensor("g1", [D], F32, kind="ExternalInput")
    be1 = nc.dram_tensor("be1", [D], F32, kind="ExternalInput")
    g2 = nc.dram_tensor("g2", [D], F32, kind="ExternalInput")
    be2 = nc.dram_tensor("be2", [D], F32, kind="ExternalInput")
    out = nc.dram_tensor("out", [Lq, D], F32, kind="ExternalOutput")

    def bcast(dram_vec, n):
        a = dram_vec[:]
        return bass.AP(tensor=a.tensor, offset=a.offset, ap=[[0, P]] + a.ap)

    with tile.TileContext(nc) as tc:
        with (
            tc.tile_pool(name="persist", bufs=1) as persist,
            tc.tile_pool(name="small", bufs=6) as small,
            tc.tile_pool(name="gb", bufs=1) as gbp,
        ):
            ident = persist.tile([P, P], F32)
            make_identity(nc, ident)
            b1s = persist.tile([P, FT], F32)
            nc.scalar.dma_start(out=b1s, in_=b1[:].rearrange("(t p) -> p t", p=P))
            b2s = persist.tile([P, DT], F32)
            nc.scalar.dma_start(out=b2s, in_=b2[:].rearrange("(t p) -> p t", p=P))
            epst = persist.tile([P, 1], F32)
            nc.vector.memset(epst, LN_EPS)
            # new_x accumulates attention output, then becomes x1 (LN1 in
            # place), then res2 base.
            new_x = persist.tile([P, LT, D], F32)
            # incremental bn_stats for LN1: one record per (l-tile, pair)
            st1 = persist.tile([P, LT, NP, 6], F32)

            # ---------------- attention ----------------
            with (
                tc.tile_pool(name="attn_sb", bufs=1) as asb,
                tc.tile_pool(name="etp", bufs=3) as etp,
                tc.tile_pool(name="utsp", bufs=4) as utsp,
                tc.tile_pool(name="scp", bufs=2, space="PSUM") as scp,
                tc.tile_pool(name="utp", bufs=2, space="PSUM") as utp,
                tc.tile_pool(name="tpp", bufs=2, space="PSUM") as tpp,
            ):
                # x^T via the DMA xbar transpose engine: chunk t holds
                # heads 2t (partitions 0-63) and 2t+1 (partitions 64-127).
                # First on the sync queue: the whole kernel starts here.
                xT = asb.tile([P, DT, S], BF16)
                for t in range(DT):
                    nc.sync.dma_start(
                        out=xT[:, t, :],
                        in_=xb16[:, t * P:(t + 1) * P],
                        transpose=True)

                # values interleaved with ones columns, host-prepacked:
                # [P, ST, H, W]; issued on the scalar queue so they don't
                # delay the xT transposes.
                vaug = asb.tile([P, ST, H * W], BF16)
                nc.scalar.dma_start(
                    out=vaug,
                    in_=xbaug[:, :].rearrange("(u p) hw -> p u hw", p=P))
                vaug = vaug.rearrange("p u (h w) -> p u h w", w=W)

                # residual rows (queries only), fp32: one DMA on the sync
                # queue behind the xT transposes (needed ~40us in).
                xres = asb.tile([P, LT, D], F32)
                nc.sync.dma_start(
                    out=xres,
                    in_=xb[0:Lq, :].rearrange("(l p) d -> p l d", p=P))

                def make_epilogue(t, s, utA, utB):
                    """Chunked epilogue for (pair, slab): returns a list of
                    closures, sprinkled across the next slab's u-loop so the
                    DVE-gated PE transposes never bubble the PE."""
                    hA, hB = 2 * t, 2 * t + 1
                    utss = {}

                    def copies():
                        for h, ut in ((hA, utA), (hB, utB)):
                            uts = utsp.tile([W, SL], F32, tag="uts",
                                            name="uts")
                            nc.vector.tensor_copy(out=uts, in_=ut)
                            utss[h] = uts

                    def tp_chunk(h, j):
                        def run():
                            lt = s * LPS + j
                            up = tpp.tile([P, W], F32, tag="tp", name="up")
                            nc.tensor.transpose(
                                up, utss[h][:, j * P:(j + 1) * P],
                                ident[0:W, 0:W])
                            nc.vector.reciprocal(
                                out=up[:, E:W], in_=up[:, E:W])
                            nc.vector.tensor_scalar_mul(
                                out=new_x[:, lt, h * E:(h + 1) * E],
                                in0=up[:, 0:E], scalar1=up[:, E:W])
                        return run

                    def res_chunk(j):
                        def run():
                            lt = s * LPS + j
                            cols = slice(t * P, (t + 1) * P)
                            nc.vector.tensor_add(
                                out=new_x[:, lt, cols],
                                in0=new_x[:, lt, cols],
                                in1=xres[:, lt, cols])
                            nc.vector.bn_stats(
                                out=st1[:, lt, t, :],
                                in_=new_x[:, lt, cols])
                        return run

                    steps = [copies]
                    for j in range(LPS):
                        steps.append(tp_chunk(hA, j))
                        steps.append(tp_chunk(hB, j))
                    for j in range(LPS):
                        steps.append(res_chunk(j))
                    return steps

                pend_epi = []
                for t in range(NP):
                    hA, hB = 2 * t, 2 * t + 1
                    for s in range(NSL):
                        ls = slice(s * SL, (s + 1) * SL)
                        utA = utp.tile([W, SL], F32, tag="ut")
                        utB = utp.tile([W, SL], F32, tag="ut")
                        pend_av = None
                        for u in range(ST):
                            sc = scp.tile([P, 2 * SL], F32)
                            et = etp.tile([P, 2 * SL], BF16)
                            ku = slice(u * P, (u + 1) * P)
                            # paired scores: rows 0-63 (head A) and 64-127
                            # (head B) of the PE run concurrently.
                            nc.tensor.matmul(
                                sc[:, 0:SL],
                                xT[0:E, t, ku], xT[0:E, t, ls],
                                start=True, stop=True)
                            nc.tensor.matmul(
                                sc[:, SL:2 * SL],
                                xT[E:P, t, ku], xT[E:P, t, ls],
                                start=True, stop=True)
                            nc.scalar.activation(
                                out=et, in_=sc, func=EXP, scale=1.0 / 8.0)
                            # drip the previous slab's epilogue into this
                            # u-loop (one step per u, starting at u=1)
                            if u >= 1 and pend_epi:
                                pend_epi.pop(0)()
                            if pend_av is not None:
                                eo, uo = pend_av
                                nc.tensor.matmul(
                                    utA, vaug[:, uo, hA, :], eo[:, 0:SL],
                                    start=(uo == 0), stop=(uo == ST - 1))
                                nc.tensor.matmul(
                                    utB, vaug[:, uo, hB, :],
                                    eo[:, SL:2 * SL],
                                    start=(uo == 0), stop=(uo == ST - 1))
                            pend_av = (et, u)
                        eo, uo = pend_av
                        nc.tensor.matmul(
                            utA, vaug[:, uo, hA, :], eo[:, 0:SL],
                            start=(uo == 0), stop=(uo == ST - 1))
                        nc.tensor.matmul(
                            utB, vaug[:, uo, hB, :], eo[:, SL:2 * SL],
                            start=(uo == 0), stop=(uo == ST - 1))
                        for step in pend_epi:  # leftovers, if any
                            step()
                        pend_epi = make_epilogue(t, s, utA, utB)
                for step in pend_epi:
                    step()

                # residual 1 + LN1 (in place on new_x)
                g1b = gbp.tile([P, D], F32, tag="g")
                nc.gpsimd.dma_start(out=g1b, in_=bcast(g1, D))
                be1b = gbp.tile([P, D], F32, tag="be")
                nc.gpsimd.dma_start(out=be1b, in_=bcast(be1, D))

            # ---------------- FFN ----------------
            with (
                tc.tile_pool(name="ffn_sb", bufs=1) as fsb,
                tc.tile_pool(name="htp", bufs=33) as htp,
                tc.tile_pool(name="w1p", bufs=3) as w1p,
                tc.tile_pool(name="w2p", bufs=2) as w2p,
                tc.tile_pool(name="ytp", bufs=3) as ytp,
                tc.tile_pool(name="ytTp", bufs=2) as ytTp,
                tc.tile_pool(name="outp", bufs=2) as outp,
                tc.tile_pool(name="ydp", bufs=1, space="DRAM") as ydp,
                tc.tile_pool(name="hpp", bufs=3, space="PSUM") as hpp,
                tc.tile_pool(name="ypp", bufs=3, space="PSUM") as ypp,
                tc.tile_pool(name="x1tp", bufs=2, space="PSUM") as x1tp,
            ):
                x1T = fsb.tile([P, DT, Lq], BF16)
                ydram = ydp.tile([D, Lq], BF16)
                st2 = fsb.tile([P, LT, 2, 6], F32)
                rstds = fsb.tile([P, LT], F32)

                def ln1_block(lts):
                    """Normalize (no affine: g1/be1 are folded into w1/b1
                    on the host) + PE-transpose to x1T.  Runs while the PE
                    is otherwise idle, so keep the chain minimal."""
                    for lt in lts:
                        mv = small.tile([P, 2], F32, tag="bnmv", name="mv")
                        nc.vector.bn_aggr(out=mv, in_=st1[:, lt, :, :])
                        nc.scalar.activation(
                            out=rstds[:, lt:lt + 1], in_=mv[:, 1:2],
                            func=SQRT, bias=epst)
                        nc.vector.reciprocal(
                            out=rstds[:, lt:lt + 1], in_=rstds[:, lt:lt + 1])
                        nc.vector.tensor_scalar(
                            out=new_x[:, lt, :], in0=new_x[:, lt, :],
                            scalar1=mv[:, 0:1], scalar2=rstds[:, lt:lt + 1],
                            op0=SUB, op1=MUL)
                        for c in range(DT):
                            tp = x1tp.tile([P, P], F32, name="tp")
                            nc.tensor.transpose(
                                tp, new_x[:, lt, c * P:(c + 1) * P], ident)
                            cp = (nc.scalar.copy if (c % 2) else
                                  nc.vector.tensor_copy)
                            cp(out=x1T[:, c, lt * P:(lt + 1) * P], in_=tp)

                def ln1_affine(lts):
                    """Apply x1 = z*g1 + be1 in place on new_x (needed only
                    for the residual-2 path); runs on the otherwise-idle
                    GPSIMD so the DVE stays free for FFN2 and the tails."""
                    for lt in lts:
                        nc.gpsimd.tensor_mul(
                            out=new_x[:, lt, :], in0=new_x[:, lt, :],
                            in1=g1b)
                        nc.gpsimd.tensor_add(
                            out=new_x[:, lt, :], in0=new_x[:, lt, :],
                            in1=be1b)

                def ffn1(s):
                    ls = slice(s * SL, (s + 1) * SL)
                    hts = []
                    for ft in range(FT):
                        w1t = w1p.tile([P, DT, P], BF16, tag="w1",
                                       name="w1t")
                        nc.sync.dma_start(out=w1t, in_=w1s[:, ft, :, :])
                        hp = hpp.tile([P, SL], F32, name="hp")
                        for dc in range(DT):
                            nc.tensor.matmul(
                                hp, w1t[:, dc, :], x1T[:, dc, ls],
                                start=(dc == 0), stop=(dc == DT - 1))
                        ht = htp.tile([P, SL], BF16, tag="ht", name="ht")
                        nc.scalar.activation(
                            out=ht, in_=hp, func=RELU,
                            bias=b1s[:, ft:ft + 1])
                        hts.append(ht)
                    return hts

                def ffn2(s, hts):
                    ls = slice(s * SL, (s + 1) * SL)
                    for dt in range(DT):
                        w2t = w2p.tile([P, FT, P], BF16, tag="w2",
                                       name="w2t")
                        nc.sync.dma_start(out=w2t, in_=w2s[:, dt, :, :])
                        yp = ypp.tile([P, SL], F32, name="yp")
                        for j in range(FT):
                            nc.tensor.matmul(
                                yp, w2t[:, j, :], hts[j],
                                start=(j == 0), stop=(j == FT - 1))
                        yTb = ytp.tile([P, SL], BF16, tag="yT", name="yTb")
                        # bias-add on the scalar engine: keeps the PSUM
                        # rotation off the DVE FIFO (busy with the tails)
                        nc.scalar.add(out=yTb, in_=yp, add=b2s[:, dt:dt + 1])
                        nc.sync.dma_start(
                            out=ydram[dt * P:(dt + 1) * P, ls], in_=yTb)

                def tail(s):
                    # y^T back to row-major via one big xbar transpose per
                    # l-tile, issued on the scalar queue; then res2 + LN2.
                    for j in range(LPS):
                        lt = s * LPS + j
                        ytT = ytTp.tile([P, D], BF16, tag="ytT", name="ytT")
                        nc.sync.dma_start(
                            out=ytT, in_=ydram[:, lt * P:(lt + 1) * P],
                            transpose=True)
                        nc.vector.tensor_add(
                            out=new_x[:, lt, :], in0=new_x[:, lt, :],
                            in1=ytT)
                        nc.vector.bn_stats(
                            out=st2[:, lt, 0, :], in_=new_x[:, lt, 0:D // 2])
                        nc.vector.bn_stats(
                            out=st2[:, lt, 1, :], in_=new_x[:, lt, D // 2:D])
                        ot = outp.tile([P, D], F32, name="ot")
                        _ln_apply(nc, small, new_x[:, lt, :],
                                  st2[:, lt, :, :], g2b, be2b, epst,
                                  out_ap=ot)
                        nc.sync.dma_start(
                            out=out[lt * P:(lt + 1) * P, :], in_=ot)

                ln1_block(range(0, LPS))
                hts0 = ffn1(0)
                ln1_block(range(LPS, LT))
                ln1_affine(range(0, LPS))
                g2b = gbp.tile([P, D], F32, tag="g")
                nc.gpsimd.dma_start(out=g2b, in_=bcast(g2, D))
                be2b = gbp.tile([P, D], F32, tag="be")
                nc.gpsimd.dma_start(out=be2b, in_=bcast(be2, D))
                ffn2(0, hts0)
                hts1 = ffn1(1)
                ln1_affine(range(LPS, LT))
                tail(0)
                ffn2(1, hts1)
                tail(1)

    nc.finalize()
    return nc


def _ln_apply(nc, small, x_ap, st_ap, gb, beb, epst, out_ap=None):
    """LayerNorm apply given pre-computed bn_stats records st_ap
    ([P, ngroups, 6]).  In place on x_ap unless out_ap given."""
    if out_ap is None:
        out_ap = x_ap
    mv = small.tile([P, 2], F32, tag="bnmv")
    nc.vector.bn_aggr(out=mv, in_=st_ap)
    rstd = small.tile([P, 1], F32, tag="rstd")
    nc.scalar.activation(out=rstd, in_=mv[:, 1:2], func=SQRT, bias=epst)
    nc.vector.reciprocal(out=rstd, in_=rstd)
    nc.vector.tensor_scalar(
        out=x_ap, in0=x_ap, scalar1=mv[:, 0:1], scalar2=rstd,
        op0=SUB, op1=MUL)
    nc.vector.tensor_mul(out=x_ap, in0=x_ap, in1=gb)
    nc.vector.tensor_add(out=out_ap, in0=x_ap, in1=beb)


# ---------------------------------------------------------------------------
# host side
# ---------------------------------------------------------------------------

_PROG_CACHE = {}


def get_program(S=2048, D=1024, F=4096):
    key = (S, D, F)
    if key not in _PROG_CACHE:
        _PROG_CACHE[key] = build_program(S, D, F)
    return _PROG_CACHE[key]


def make_in_maps(x, w1, b1, w2, b2, g1, be1, g2, be2, n_cores=8):
    B, L, D = x.shape
    F = w1.shape[0]
    H = D // E
    Lq = L // 2
    DT, FT = D // 128, F // 128
    import ml_dtypes
    BF = ml_dtypes.bfloat16
    # Fold LN1's affine into the first FFN layer: the device feeds the
    # *normalized* x into FFN1 and applies g1/be1 only on the residual
    # path.  relu(w1 @ (z*g1 + be1) + b1) == relu((w1*g1) @ z + (b1 + w1@be1))
    b1 = b1 + w1 @ be1
    w1 = w1 * g1[None, :]
    # w1s[p_d, ft, dc, p_f] so one DMA per f-tile has 2KB/partition lines
    w1s = np.ascontiguousarray(
        w1.T.reshape(DT, 128, FT, 128).transpose(1, 2, 0, 3)).astype(BF)
    # w2s[p_f, dt, ft, p_d] so one DMA per d-tile has 8KB/partition lines
    w2s = np.ascontiguousarray(
        w2.T.reshape(FT, 128, DT, 128).transpose(1, 2, 0, 3)).astype(BF)
    common = dict(w1s=w1s, w2s=w2s, b1=b1, b2=b2, g1=g1, be1=be1,
                  g2=g2, be2=be2)
    in_maps = []
    for c in range(n_cores):
        b, half = c // 2, c % 2
        lo = half * Lq
        xq = x[b, lo:lo + Lq]
        xo = x[b, Lq - lo:2 * Lq - lo]
        xbl = np.ascontiguousarray(np.concatenate([xq, xo], axis=0))
        xbl16 = xbl.astype(BF)
        # values + ones column per head, host-baked: [S, H, W]
        aug = np.empty((L, H, E + 1), dtype=BF)
        aug[:, :, 0:E] = xbl16.reshape(L, H, E)
        aug[:, :, E] = BF(1.0)
        in_maps.append(dict(xb=xbl, xb16=xbl16,
                            xbaug=np.ascontiguousarray(aug.reshape(L, -1)),
                            **common))
    return in_maps


def kernel(x, w1, b1, w2, b2, g1, be1, g2, be2):
    from concourse.bass_utils import run_bass_kernel_spmd

    x = np.asarray(x, dtype=np.float32)
    B, L, D = x.shape
    F = w1.shape[0]
    Lq = L // 2
    n_cores = 2 * B
    nc = get_program(L, D, F)
    in_maps = make_in_maps(x, np.asarray(w1, np.float32), np.asarray(b1, np.float32),
                           np.asarray(w2, np.float32), np.asarray(b2, np.float32),
                           np.asarray(g1, np.float32), np.asarray(be1, np.float32),
                           np.asarray(g2, np.float32), np.asarray(be2, np.float32),
                           n_cores)
    res = run_bass_kernel_spmd(nc, in_maps, core_ids=list(range(n_cores)))
    outp = np.empty((B, L, D), dtype=np.float32)
    for c in range(n_cores):
        b, half = c // 2, c % 2
        outp[b, half * Lq:(half + 1) * Lq] = res.results[c]["out"]
    return outp
